# revision 1
# baseline (speedup 1.0000x reference)
"""DSS layer (S4-style diagonal state space) Trainium2 kernel.

Full inputs:  u [8,128,4096], Lambda_re/im [128,64], W_ri [128,64,2],
              D [128], log_step [128]  ->  y [8,128,4096]

Sharding: H split 8 ways (16 channels/core), all B on every core.

Algorithm (chunked semiseparable scan, C=256, T=16 chunks):
  With z = exp(step*Lambda) (|z|<1 since Re(Lambda)<0, so the reference's
  complex-softmax argmax is always at l=0), the DSS kernel is
      K[l] = Re sum_n c_n z_n^l,   c = (W/Lambda)*conj(s)/(|s|^2+eps),
      s = (1-z^L)/(1-z)
  and y = causal_conv(K, u) + D*u. Per (b,h) the conv splits into
    intra-chunk:  dense Toeplitz matmul; the shifted-Toeplitz operand
                  kshift[j,i] = K[i-j] (+ D on the diagonal) is itself
                  built on the PE as Re(z^{-j} * c z^i) + causal mask
    inter-chunk:  states s[J] = sum_j z^{C-1-j} u[J*C+j]  (matmul),
                  scan h[I] = z^C h[I-1] + s[I-1]          (vector),
                  y_inter = Re sum_n c z^{i+1} h_n[I]      (matmul).

Everything runs on-device except O(H*N) parameter prep (z, c, broadcast
tables) done host-side in float64.
"""
import numpy as np

import concourse.bass as bass
import concourse.bacc as bacc
import concourse.tile as tile
from concourse import mybir
from concourse.bass_utils import run_bass_kernel_spmd
from concourse.masks import make_identity

F32 = mybir.dt.float32
I32 = mybir.dt.int32
PI = float(np.pi)

B, H, L, N = 8, 128, 4096, 64
NCORES = 8
HL = H // NCORES            # 16 channels per core
BH = B * HL                 # 128 partition rows
C = 256                     # chunk length
T = L // C                  # 16 chunks
NB = L // 128               # 32 position blocks
NQ = HL // 2                # 8 h-pairs
EPS = 1e-7

_CACHE = {}


def _build(debug=False):
    """Build the SPMD Bass program (identical on all cores)."""
    nc = bacc.Bacc(trn_type="TRN2", target_bir_lowering=False)

    # ---------------- DRAM I/O (per core) ----------------
    # u pre-transposed on host: u_tk[p, h*128+J*8+b] = u[b, h, (2J+k)*128+p]
    ut0_d = nc.dram_tensor("u_t0", [128, T * 128], F32, kind="ExternalInput")
    ut1_d = nc.dram_tensor("u_t1", [128, T * 128], F32, kind="ExternalInput")
    a2_d = nc.dram_tensor("a2", [128, NQ], F32, kind="ExternalInput")
    f2_d = nc.dram_tensor("f2", [128, NQ], F32, kind="ExternalInput")
    lc2_d = nc.dram_tensor("lc2", [128, NQ], F32, kind="ExternalInput")
    g2_d = nc.dram_tensor("g2", [128, NQ], F32, kind="ExternalInput")
    gc2_d = nc.dram_tensor("gc2", [128, NQ], F32, kind="ExternalInput")
    na2_d = nc.dram_tensor("na2", [128, NQ], F32, kind="ExternalInput")
    arow_d = nc.dram_tensor("arow", [1, HL * N], F32, kind="ExternalInput")
    frow_d = nc.dram_tensor("frow", [1, HL * N], F32, kind="ExternalInput")
    iota_d = nc.dram_tensor("iota_f", [128, C + 1], F32, kind="ExternalInput")
    irev_d = nc.dram_tensor("irev", [128, 2], F32, kind="ExternalInput")
    war_d = nc.dram_tensor("war", [128, 64], F32, kind="ExternalInput")
    wai_d = nc.dram_tensor("wai", [128, 64], F32, kind="ExternalInput")
    w2r_d = nc.dram_tensor("w2r", [128, 64], F32, kind="ExternalInput")
    w2i_d = nc.dram_tensor("w2i", [128, 64], F32, kind="ExternalInput")
    war8_d = nc.dram_tensor("war8", [128, 512], F32, kind="ExternalInput")
    wai8_d = nc.dram_tensor("wai8", [128, 512], F32, kind="ExternalInput")
    dcol_d = nc.dram_tensor("dcol", [1, HL], F32, kind="ExternalInput")

    # rows = I*8+b, cols = h*C+c  (host un-permutes)
    y_d = nc.dram_tensor("y_s", [128, HL * C], F32, kind="ExternalOutput")
    dbg = {}
    if debug:
        for nm, shp in [("SSr", [128, 1024]), ("HSr", [128, 1024]),
                        ("kshift0", [128, 256]), ("ysb0", [128, 256])]:
            dbg[nm] = nc.dram_tensor("dbg_" + nm, shp, F32,
                                     kind="ExternalOutput")

    with tile.TileContext(nc) as tc, \
         tc.tile_pool(name="big", bufs=1) as big, \
         tc.tile_pool(name="tabs", bufs=1) as tabs, \
         tc.tile_pool(name="small", bufs=1) as small, \
         tc.tile_pool(name="work", bufs=2) as work, \
         tc.tile_pool(name="workbig", bufs=1) as workbig, \
         tc.tile_pool(name="ps1", bufs=2, space="PSUM") as ps1, \
         tc.tile_pool(name="psk", bufs=2, space="PSUM") as psk, \
         tc.tile_pool(name="psy", bufs=4, space="PSUM") as psy:

        # ---------------- load everything ----------------
        def load(d, shape, tag):
            t = small.tile(list(shape), F32, tag=tag)
            nc.sync.dma_start(out=t, in_=d[:])
            return t

        a2 = load(a2_d, (128, NQ), "a2"); f2 = load(f2_d, (128, NQ), "f2")
        lc2 = load(lc2_d, (128, NQ), "lc2"); g2 = load(g2_d, (128, NQ), "g2")
        gc2 = load(gc2_d, (128, NQ), "gc2"); na2 = load(na2_d, (128, NQ), "na2")
        iota = load(iota_d, (128, C + 1), "iota")
        irev = load(irev_d, (128, 2), "irev")
        war = load(war_d, (128, 64), "war"); wai = load(wai_d, (128, 64), "wai")
        w2r = load(w2r_d, (128, 64), "w2r"); w2i = load(w2i_d, (128, 64), "w2i")
        war8 = load(war8_d, (128, 512), "war8")
        wai8 = load(wai8_d, (128, 512), "wai8")

        def bcast_load(d, ncols, tag):
            t = small.tile([128, ncols], F32, tag=tag)
            nc.sync.dma_start(
                out=t, in_=bass.AP(tensor=d[:].tensor, offset=0,
                                   ap=[[0, 128], [1, ncols]]))
            return t

        a_bc = bcast_load(arow_d, HL * N, "a_bc")
        f_bc = bcast_load(frow_d, HL * N, "f_bc")
        d_bc = bcast_load(dcol_d, HL, "d_bc")

        ident = small.tile([128, 128], F32)
        make_identity(nc, ident)

        TS = nc.vector.tensor_scalar
        TT = nc.vector.tensor_tensor
        MUL = mybir.AluOpType.mult
        ADD = mybir.AluOpType.add
        SUB = mybir.AluOpType.subtract

        # ---------------- u already transposed host-side ----------------
        u_T0 = big.tile([128, T * 128], F32)
        u_T1 = big.tile([128, T * 128], F32)
        uT = [u_T0, u_T1]
        nc.sync.dma_start(out=u_T0, in_=ut0_d[:])
        nc.sync.dma_start(out=u_T1, in_=ut1_d[:])

        # ================= table generation =================
        # Pass A: all args + Exp (one act table set); pass B: all Sin.
        EXP = mybir.ActivationFunctionType.Exp
        SIN = mybir.ActivationFunctionType.Sin

        # -- pass A: exponentials (one act-table set)
        _ecm = tc.tile_pool(name="expool", bufs=1)
        expool = _ecm.__enter__()
        _tcm = tc.tile_pool(name="trigp", bufs=2)
        trigp = _tcm.__enter__()
        zf_E = []
        for k in range(2):
            arg_e = work.tile([128, HL * N], F32, tag="zfarg")
            TS(out=arg_e, in0=a_bc, scalar1=irev[:, k:k + 1],
               scalar2=None, op0=MUL)
            E = expool.tile([128, HL * N], F32, tag=f"zfE{k}")
            nc.scalar.activation(out=E, in_=arg_e, func=EXP)
            zf_E.append(E)
        eziE = []
        for q in range(NQ):
            arg_e = work.tile([128, C + 1 + 128], F32, tag="earg")
            TS(out=arg_e[:, 0:C + 1], in0=iota, scalar1=a2[:, q:q + 1],
               scalar2=lc2[:, q:q + 1], op0=MUL, op1=ADD)
            TS(out=arg_e[:, C + 1:], in0=iota[:, 0:128],
               scalar1=na2[:, q:q + 1], scalar2=None, op0=MUL)
            E = expool.tile([128, C + 1 + 128], F32, tag=f"eziE{q}")
            nc.scalar.activation(out=E, in_=arg_e, func=EXP)
            eziE.append(E)

        # -- pass B: merged fracs + one Sin per tile-family + assemble
        # sign tricks (scale=+2pi): -sin via +0.5 offset, +cos via +0.25
        def trig(out_t, arg_t, eng="vector"):
            e = getattr(nc, eng)
            shp = [arg_t.shape[0], arg_t.shape[1]]
            pl = expool if shp[1] > 1024 else trigp
            ai = pl.tile(shp, I32, tag=f"trig_i{shp[1]}")
            e.tensor_copy(ai, arg_t)
            fr = pl.tile(shp, F32, tag=f"trig_f{shp[1]}")
            e.tensor_tensor(out=fr, in0=arg_t, in1=ai, op=SUB)
            nc.scalar.activation(out=out_t, in_=fr, func=SIN,
                                 bias=0.0, scale=2.0 * PI)
            return out_t

        # zf: sections [sin | cos]; zf_k cols = h*128 + ri*64 + n
        zft = []
        for k in range(2):
            args = expool.tile([128, 2 * HL * N], F32, tag="zfargs")
            nc.gpsimd.tensor_scalar(out=args[:, 0:HL * N], in0=f_bc,
                                    scalar1=irev[:, k:k + 1],
                                    scalar2=None, op0=MUL)
            nc.gpsimd.tensor_scalar(out=args[:, HL * N:], in0=args[:, 0:HL * N],
                                    scalar1=0.25, scalar2=None, op0=ADD)
            TR = expool.tile([128, 2 * HL * N], F32, tag="zfTR")
            trig(TR, args, eng="gpsimd")
            tab = tabs.tile([128, 2 * HL * N], F32, tag=f"zft{k}")
            tv = tab.rearrange("p (h r n) -> p h r n", h=HL, r=2)
            Ev = zf_E[k].rearrange("p (h n) -> p h n", h=HL)
            COv = TR[:, HL * N:].rearrange("p (h n) -> p h n", h=HL)
            SNv = TR[:, 0:HL * N].rearrange("p (h n) -> p h n", h=HL)
            TT(out=tv[:, :, 0, :], in0=Ev, in1=COv, op=MUL)
            TT(out=tv[:, :, 1, :], in0=Ev, in1=SNv, op=MUL)
            zft.append(tab)

        # E + ZI per pair: sections [E-msin | E-cos | ZI-msin | ZI-cos]
        # host supplies g2 = frac(g + 0.5) (for -sin), gc2 = frac(g + 0.25)
        W1, W2 = C + 1, 2 * (C + 1)
        W3, W4 = W2 + 128, W2 + 256
        E2a = []; E2b = []; ZIr = []; ZIi = []
        for q in range(NQ):
            args = trigp.tile([128, W4], F32, tag="eziargs")
            TS(out=args[:, 0:W1], in0=iota, scalar1=f2[:, q:q + 1],
               scalar2=g2[:, q:q + 1], op0=MUL, op1=ADD)
            TS(out=args[:, W1:W2], in0=iota, scalar1=f2[:, q:q + 1],
               scalar2=gc2[:, q:q + 1], op0=MUL, op1=ADD)
            TS(out=args[:, W2:W3], in0=iota[:, 0:128],
               scalar1=f2[:, q:q + 1], scalar2=0.5, op0=MUL, op1=ADD)
            TS(out=args[:, W3:W4], in0=iota[:, 0:128],
               scalar1=f2[:, q:q + 1], scalar2=0.25, op0=MUL, op1=ADD)
            TR = trigp.tile([128, W4], F32, tag="eziTR")
            trig(TR, args)
            a_t = tabs.tile([128, C + 1], F32, tag=f"E2a{q}")
            b_t = tabs.tile([128, C + 1], F32, tag=f"E2b{q}")
            r_t = tabs.tile([128, 128], F32, tag=f"ZIr{q}")
            i_t = tabs.tile([128, 128], F32, tag=f"ZIi{q}")
            TT(out=a_t, in0=eziE[q][:, 0:W1], in1=TR[:, W1:W2], op=MUL)
            TT(out=b_t, in0=eziE[q][:, 0:W1], in1=TR[:, 0:W1], op=MUL)
            TT(out=r_t, in0=eziE[q][:, W1:], in1=TR[:, W3:W4], op=MUL)
            TT(out=i_t, in0=eziE[q][:, W1:], in1=TR[:, W2:W3], op=MUL)
            E2a.append(a_t); E2b.append(b_t)
            ZIr.append(r_t); ZIi.append(i_t)
        _tcm.__exit__(None, None, None)
        _ecm.__exit__(None, None, None)

        # ---------------- stage 1: chunk states ----------------------------
        # SSr/SSi rows (h%2)*64+n, cols (h//2)*128 + J*8 + b
        SSr = big.tile([128, NQ * 128], F32)
        SSi = big.tile([128, NQ * 128], F32)
        for h in range(HL):
            q, par = h // 2, h % 2
            ps_s = ps1.tile([128, 128], F32)
            for k in range(2):
                nc.tensor.matmul(ps_s, zft[k][:, h * 128:(h + 1) * 128],
                                 uT[k][:, h * 128:(h + 1) * 128],
                                 start=(k == 0), stop=(k == 1))
            sl = slice(par * 64, (par + 1) * 64)
            # cols (J, q, b): J*64 + q*8 + b
            dst_r = SSr.rearrange("p (j q b) -> p j q b",
                                  j=T, q=NQ)[sl, :, q, :]
            dst_i = SSi.rearrange("p (j q b) -> p j q b",
                                  j=T, q=NQ)[sl, :, q, :]
            srcv = ps_s.rearrange("p (j b) -> p j b", j=T)
            nc.vector.tensor_copy(dst_r, srcv[0:64])
            nc.vector.tensor_copy(dst_i, srcv[64:128])

        # ---------------- kshift construction (PE) --------------------------
        # kshift_h[p, m] = Re(z^-p * c z^m) + D[h]*delta[p,m], masked m>=p
        _kcm = tc.tile_pool(name="kpool", bufs=1)
        kpool = _kcm.__enter__()
        _ycm = tc.tile_pool(name="ypool", bufs=1)
        ypool = _ycm.__enter__()
        y_stage = ypool.tile([128, HL * C], F32, tag="ystage")
        kshifts = []
        for h in range(HL):
            q, par = h // 2, h % 2
            sl = slice(par * 64, (par + 1) * 64)
            ps_k = psk.tile([128, C], F32)
            nc.tensor.matmul(ps_k, ZIr[q][sl, :], E2a[q][sl, 0:C],
                             start=True, stop=False)
            nc.tensor.matmul(ps_k, ZIi[q][sl, :], E2b[q][sl, 0:C],
                             start=False, stop=False)
            dident = work.tile([128, 128], F32, tag="dident")
            TS(out=dident, in0=ident, scalar1=d_bc[:, h:h + 1],
               scalar2=None, op0=MUL)
            nc.tensor.matmul(ps_k[:, 0:128], dident, ident,
                             start=False, stop=True)
            ksb = kpool.tile([128, C], F32, tag=f"kshift{h}")
            nc.scalar.copy(ksb, ps_k)
            # causal mask: zero where m < p
            nc.gpsimd.affine_select(
                out=ksb, in_=ksb, compare_op=mybir.AluOpType.is_ge,
                base=0, channel_multiplier=-1, pattern=[[1, C]], fill=0.0)
            kshifts.append(ksb)
            if debug and h == 0:
                nc.sync.dma_start(out=dbg["kshift0"][:], in_=ksb)

        # ---------------- scan over chunks ---------------------------------
        HSr = big.tile([128, NQ * 128], F32)
        HSi = big.tile([128, NQ * 128], F32)
        nc.vector.memset(HSr, 0.0)
        nc.vector.memset(HSi, 0.0)

        def step_sl(t, I):
            return t[:, I * 64:(I + 1) * 64]

        def ev(t, n=7, off=0):   # [128, n, 64] view of even slices (+off)
            return t.rearrange("p (j2 two x) -> p j2 two x",
                               j2=T // 2, two=2)[:, 0:n, off, :]

        # G[k] = w (.) SS[2k] + SS[2k+1], k = 0..6   (8 ops, [128, 7, 64])
        NK = T // 2 - 1
        Gr = big.tile([128, NK * 64], F32)
        Gi = big.tile([128, NK * 64], F32)
        w8r = war8.rearrange("p (j2 x) -> p j2 x", j2=T // 2)[:, 0:NK, :]
        w8i = wai8.rearrange("p (j2 x) -> p j2 x", j2=T // 2)[:, 0:NK, :]
        Grv = Gr.rearrange("p (k x) -> p k x", k=NK)
        Giv = Gi.rearrange("p (k x) -> p k x", k=NK)
        gm1 = workbig.tile([128, NK * 64], F32, tag="gm1")
        gm1v = gm1.rearrange("p (k x) -> p k x", k=NK)
        gm2 = workbig.tile([128, NK * 64], F32, tag="gm2")
        gm2v = gm2.rearrange("p (k x) -> p k x", k=NK)
        TT(out=gm1v, in0=w8r, in1=ev(SSr), op=MUL)
        TT(out=gm2v, in0=w8i, in1=ev(SSi), op=MUL)
        TT(out=gm1v, in0=gm1v, in1=gm2v, op=SUB)
        TT(out=Grv, in0=gm1v, in1=ev(SSr, off=1), op=ADD)
        TT(out=gm1v, in0=w8i, in1=ev(SSr), op=MUL)
        TT(out=gm2v, in0=w8r, in1=ev(SSi), op=MUL)
        TT(out=gm1v, in0=gm1v, in1=gm2v, op=ADD)
        TT(out=Giv, in0=gm1v, in1=ev(SSi, off=1), op=ADD)

        # even chain: h[2k+2] = w^2 (.) h[2k] + G[k]   (7 steps x 8 ops)
        for k in range(NK):
            pr, pi = step_sl(HSr, 2 * k), step_sl(HSi, 2 * k)
            m1 = work.tile([128, 64], F32, tag="scm1")
            TT(out=m1, in0=w2r, in1=pr, op=MUL)
            m2 = work.tile([128, 64], F32, tag="scm2")
            TT(out=m2, in0=w2i, in1=pi, op=MUL)
            tr = work.tile([128, 64], F32, tag="sctr")
            TT(out=tr, in0=m1, in1=m2, op=SUB)
            TT(out=step_sl(HSr, 2 * k + 2), in0=tr,
               in1=Gr[:, k * 64:(k + 1) * 64], op=ADD)
            m3 = work.tile([128, 64], F32, tag="scm3")
            TT(out=m3, in0=w2i, in1=pr, op=MUL)
            m4 = work.tile([128, 64], F32, tag="scm4")
            TT(out=m4, in0=w2r, in1=pi, op=MUL)
            ti = work.tile([128, 64], F32, tag="scti")
            TT(out=ti, in0=m3, in1=m4, op=ADD)
            TT(out=step_sl(HSi, 2 * k + 2), in0=ti,
               in1=Gi[:, k * 64:(k + 1) * 64], op=ADD)

        # odd fill: h[2k+1] = w (.) h[2k] + SS[2k], all k at once (8 ops)
        w8rf = war8.rearrange("p (j2 x) -> p j2 x", j2=T // 2)
        w8if = wai8.rearrange("p (j2 x) -> p j2 x", j2=T // 2)
        om1 = workbig.tile([128, (T // 2) * 64], F32, tag="om1")
        om1v = om1.rearrange("p (k x) -> p k x", k=T // 2)
        om2 = workbig.tile([128, (T // 2) * 64], F32, tag="om2")
        om2v = om2.rearrange("p (k x) -> p k x", k=T // 2)
        TT(out=om1v, in0=w8rf, in1=ev(HSr, n=T // 2), op=MUL)
        TT(out=om2v, in0=w8if, in1=ev(HSi, n=T // 2), op=MUL)
        TT(out=om1v, in0=om1v, in1=om2v, op=SUB)
        TT(out=ev(HSr, n=T // 2, off=1), in0=om1v,
           in1=ev(SSr, n=T // 2), op=ADD)
        TT(out=om1v, in0=w8if, in1=ev(HSr, n=T // 2), op=MUL)
        TT(out=om2v, in0=w8rf, in1=ev(HSi, n=T // 2), op=MUL)
        TT(out=om1v, in0=om1v, in1=om2v, op=ADD)
        TT(out=ev(HSi, n=T // 2, off=1), in0=om1v,
           in1=ev(SSi, n=T // 2), op=ADD)

        # repack scan output (J, q, b) -> (q, J, b) for stage-3 lhsT slices
        HS2r = big.tile([128, NQ * 128], F32)
        HS2i = big.tile([128, NQ * 128], F32)
        nc.vector.tensor_copy(
            HS2r.rearrange("p (q j b) -> p q j b", q=NQ, j=T),
            HSr.rearrange("p (j q b) -> p q j b", j=T, q=NQ))
        nc.vector.tensor_copy(
            HS2i.rearrange("p (q j b) -> p q j b", q=NQ, j=T),
            HSi.rearrange("p (j q b) -> p q j b", j=T, q=NQ))

        if debug:
            nc.sync.dma_start(out=dbg["SSr"][:], in_=SSr)
            nc.sync.dma_start(out=dbg["HSr"][:], in_=HS2r)

        # ---------------- stage 3 + intra + output -------------------------
        for h in range(HL):
            q, par = h // 2, h % 2
            sl = slice(par * 64, (par + 1) * 64)
            cs = slice(q * 128, (q + 1) * 128)

            ps_y = psy.tile([128, C], F32)
            # intra first: independent of the scan, overlaps it on PE
            nc.tensor.matmul(ps_y, u_T0[:, h * 128:(h + 1) * 128],
                             kshifts[h][:, 0:C], start=True, stop=False)
            nc.tensor.matmul(ps_y[:, 128:C], u_T1[:, h * 128:(h + 1) * 128],
                             kshifts[h][:, 0:128], start=False, stop=False)
            nc.tensor.matmul(ps_y, HS2r[sl, cs], E2a[q][sl, 1:C + 1],
                             start=False, stop=False)
            nc.tensor.matmul(ps_y, HS2i[sl, cs], E2b[q][sl, 1:C + 1],
                             start=False, stop=True)

            nc.scalar.copy(y_stage[:, h * C:(h + 1) * C], ps_y)
            if debug and h == 0:
                nc.sync.dma_start(out=dbg["ysb0"][:],
                                  in_=y_stage[:, h * C:(h + 1) * C])

        nc.sync.dma_start(out=y_d[:], in_=y_stage)

        _ycm.__exit__(None, None, None)
        _kcm.__exit__(None, None, None)

    nc.compile()
    return nc


def _host_prep(u, Lambda_re, Lambda_im, W_ri, D, log_step):
    """Per-core input dicts (float64 param prep, O(H*N) work)."""
    u = np.asarray(u, np.float32)
    Lr = np.asarray(Lambda_re, np.float64)
    Li = np.asarray(Lambda_im, np.float64)
    W = np.asarray(W_ri, np.float64)
    Dv = np.asarray(D, np.float64)
    ls = np.asarray(log_step, np.float64)

    step = np.exp(ls)                                    # [H]
    a = step[:, None] * Lr                               # [H,N]
    th = step[:, None] * Li                              # radians
    f = np.mod(th / (2 * np.pi), 1.0)                    # cycles frac
    z = np.exp(a + 1j * th)
    w = z ** C
    zL = z ** L
    s = (1.0 - zL) / (1.0 - z)
    Lam = Lr + 1j * Li
    c = (W[..., 0] + 1j * W[..., 1]) / Lam * np.conj(s) / (s * np.conj(s) + EPS)
    mag = np.maximum(np.abs(c), 1e-30)
    lc = np.log(mag)
    g = np.mod(np.angle(c) / (2 * np.pi), 1.0)

    in_maps = []
    for core in range(NCORES):
        hs = slice(core * HL, (core + 1) * HL)

        # [B,HL,T,2,128] -> [k][p][h][J][b]
        uu = np.asarray(u[:, hs]).reshape(B, HL, T, 2, 128)
        uu_t = np.ascontiguousarray(
            uu.transpose(3, 4, 1, 2, 0)).reshape(2, 128, T * 128)
        uu_t = [np.ascontiguousarray(uu_t[0]), np.ascontiguousarray(uu_t[1])]

        def wpack(x):  # [HL,N] -> [(par,n), (q,b)] replicated over b
            return np.ascontiguousarray(
                np.asarray(x).reshape(HL // 2, 2, N).transpose(1, 2, 0)
            ).reshape(128, HL // 2, 1).repeat(8, axis=2).reshape(
                128, 64).astype(np.float32)

        def pack2(x):  # [HL,N] -> [(par,n), q]
            xr = np.asarray(x[hs]).reshape(HL // 2, 2, N)     # [q, par, n]
            return np.ascontiguousarray(
                xr.transpose(1, 2, 0).reshape(128, HL // 2)).astype(np.float32)

        m = {
            "u_t0": uu_t[0], "u_t1": uu_t[1],
            "a2": pack2(a), "f2": pack2(f), "lc2": pack2(lc),
            "g2": pack2(np.mod(g + 0.5, 1.0)),
            "gc2": pack2(np.mod(g + 0.25, 1.0)),
            "na2": pack2(-a),
            "arow": a[hs].reshape(1, HL * N).astype(np.float32),
            "frow": f[hs].reshape(1, HL * N).astype(np.float32),
            "iota_f": np.broadcast_to(
                np.arange(C + 1, dtype=np.float32), (128, C + 1)).copy(),
            "irev": np.stack([255.0 - np.arange(128.0),
                              127.0 - np.arange(128.0)],
                             axis=1).astype(np.float32),
            # war/wai rows (par,n), cols (q, b): w[h=2q+par, n] repl over b
            "war": np.ascontiguousarray(
                np.real(w[hs]).reshape(HL // 2, 2, N).transpose(1, 2, 0)
            ).reshape(128, HL // 2, 1).repeat(8, axis=2).reshape(128, 64
            ).astype(np.float32),
            "wai": np.ascontiguousarray(
                np.imag(w[hs]).reshape(HL // 2, 2, N).transpose(1, 2, 0)
            ).reshape(128, HL // 2, 1).repeat(8, axis=2).reshape(128, 64
            ).astype(np.float32),
            "w2r": wpack(np.real(w[hs] ** 2)),
            "w2i": wpack(np.imag(w[hs] ** 2)),
            "war8": np.tile(wpack(np.real(w[hs])), (1, T // 2)).reshape(
                128, T // 2, 64).reshape(128, (T // 2) * 64),
            "wai8": np.tile(wpack(np.imag(w[hs])), (1, T // 2)).reshape(
                128, T // 2, 64).reshape(128, (T // 2) * 64),
            "dcol": Dv[hs].reshape(1, HL).astype(np.float32),
        }
        in_maps.append(m)
    return in_maps


def _run(inputs, trace=False, debug=False):
    key = "nc_dbg" if debug else "nc"
    if key not in _CACHE:
        _CACHE[key] = _build(debug=debug)
    nc = _CACHE[key]
    in_maps = _host_prep(**inputs)
    res = run_bass_kernel_spmd(nc, in_maps, list(range(NCORES)), trace=trace)
    parts = []
    for core in range(NCORES):
        ys = res.results[core]["y_s"].reshape(T, B, HL, C)
        parts.append(ys.transpose(1, 2, 0, 3).reshape(B, HL, L))
    y = np.concatenate(parts, axis=1)                    # [B, H, L]
    return np.ascontiguousarray(y.astype(np.float32)), res


def kernel(**inputs) -> np.ndarray:
    y, _ = _run(inputs, trace=False)
    return y


def kernel_traced(**inputs):
    y, res = _run(inputs, trace=True)
    return y, res



# revision 2
# speedup vs baseline: 3.5356x; 3.5356x over previous
"""DSS layer (S4-style diagonal state space) Trainium2 kernel, v2.

Full inputs:  u [8,128,4096], Lambda_re/im [128,64], W_ri [128,64,2],
              D [128], log_step [128]  ->  y [8,128,4096]

Sharding: H split 8 ways (16 channels/core), all B on every core.

Algorithm (chunked semiseparable scan, C=256, T=16 chunks):
  With z = exp(step*Lambda) (|z|<1 since Re(Lambda)<0, so the reference's
  complex-softmax argmax is always at l=0), the DSS kernel is
      K[l] = Re sum_n c_n z_n^l,   c = (W/Lambda)*conj(s)/(|s|^2+eps),
      s = (1-z^L)/(1-z)
  and y = causal_conv(K, u) + D*u. Per (b,h) the conv splits into
    intra-chunk:  dense Toeplitz matmul with kshift[j,i] = K[i-j]
                  (+ D on the diagonal), causal-masked
    inter-chunk:  states s[J] = sum_j z^{C-1-j} u[J*C+j]  (matmul),
                  scan h[I] = z^C h[I-1] + s[I-1]          (vector),
                  y_inter = Re sum_n c z^{i+1} h_n[I]      (matmul).

v2: all O(H*N*C) tables (z-power stage-1 weights, Toeplitz kshift,
stage-3 c*z^i tables) are precomputed host-side in float64 and DMA'd in
as bf16; every matmul runs in bf16 (4x PE rate vs fp32). The scan stays
fp32 on the vector engine. Stage-3 intra matmuls are issued before the
scan-dependent inter matmuls so the PE overlaps the vector scan.
"""
import numpy as np
import ml_dtypes

import concourse.bass as bass
import concourse.bacc as bacc
import concourse.tile as tile
from concourse import mybir
from concourse.bass_utils import run_bass_kernel_spmd

F32 = mybir.dt.float32
BF16 = mybir.dt.bfloat16
BF16_NP = ml_dtypes.bfloat16

B, H, L, N = 8, 128, 4096, 64
NCORES = 8
HL = H // NCORES            # 16 channels per core
C = 256                     # chunk length
T = L // C                  # 16 chunks
NQ = HL // 2                # 8 h-pairs
EPS = 1e-7

_CACHE = {}


def _build():
    """Build the SPMD Bass program (identical on all cores)."""
    nc = bacc.Bacc(trn_type="TRN2", target_bir_lowering=False)

    # ---------------- DRAM I/O (per core) ----------------
    # ut_k[p, h*128+J*8+b] = u[b, h, (2J+k)*128+p]
    ut0_d = nc.dram_tensor("ut0", [128, T * 128], BF16, kind="ExternalInput")
    ut1_d = nc.dram_tensor("ut1", [128, T * 128], BF16, kind="ExternalInput")
    # zft_k[p, h*128+r*64+n] = Re/Im(z^{(255|127)-p})
    zft0_d = nc.dram_tensor("zft0", [128, HL * 128], BF16, kind="ExternalInput")
    zft1_d = nc.dram_tensor("zft1", [128, HL * 128], BF16, kind="ExternalInput")
    # ksh[p, h*C+m] = K[h,m-p] (m>p), K[h,0]+D[h] (m=p), 0 (m<p)
    ksh_d = nc.dram_tensor("ksh", [128, HL * C], BF16, kind="ExternalInput")
    # e2a[par*64+n, q*257+j] = Re(c z^j), e2b = -Im(c z^j), h=2q+par
    e2a_d = nc.dram_tensor("e2a", [128, NQ * (C + 1)], BF16,
                           kind="ExternalInput")
    e2b_d = nc.dram_tensor("e2b", [128, NQ * (C + 1)], BF16,
                           kind="ExternalInput")
    # scan tables (fp32), rows (par,n), cols (q,b) [repl over b]
    w2r_d = nc.dram_tensor("w2r", [128, 64], F32, kind="ExternalInput")
    w2i_d = nc.dram_tensor("w2i", [128, 64], F32, kind="ExternalInput")
    war8_d = nc.dram_tensor("war8", [128, 512], F32, kind="ExternalInput")
    wai8_d = nc.dram_tensor("wai8", [128, 512], F32, kind="ExternalInput")

    # rows = J*8+b, cols = h*C+c  (host un-permutes)
    y_d = nc.dram_tensor("y_s", [128, HL * C], F32, kind="ExternalOutput")

    with tile.TileContext(nc) as tc, \
         tc.tile_pool(name="big", bufs=1) as big, \
         tc.tile_pool(name="work", bufs=2) as work, \
         tc.tile_pool(name="workbig", bufs=1) as workbig, \
         tc.tile_pool(name="ps1", bufs=2, space="PSUM") as ps1, \
         tc.tile_pool(name="psy", bufs=6, space="PSUM") as psy:

        def load(d, shape, dt, tag):
            t = big.tile(list(shape), dt, tag=tag)
            nc.sync.dma_start(out=t, in_=d[:])
            return t

        # stage-1 operands first so its matmuls start ASAP
        zft0 = load(zft0_d, (128, HL * 128), BF16, "zft0")
        ut0 = load(ut0_d, (128, T * 128), BF16, "ut0")
        zft1 = load(zft1_d, (128, HL * 128), BF16, "zft1")
        ut1 = load(ut1_d, (128, T * 128), BF16, "ut1")
        ksh = load(ksh_d, (128, HL * C), BF16, "ksh")
        w2r = load(w2r_d, (128, 64), F32, "w2r")
        w2i = load(w2i_d, (128, 64), F32, "w2i")
        war8 = load(war8_d, (128, 512), F32, "war8")
        wai8 = load(wai8_d, (128, 512), F32, "wai8")
        e2a = load(e2a_d, (128, NQ * (C + 1)), BF16, "e2a")
        e2b = load(e2b_d, (128, NQ * (C + 1)), BF16, "e2b")

        TT = nc.vector.tensor_tensor
        MUL = mybir.AluOpType.mult
        ADD = mybir.AluOpType.add
        SUB = mybir.AluOpType.subtract
        uT = [ut0, ut1]
        zft = [zft0, zft1]

        # ---------------- stage 1: chunk states ----------------------------
        # SSr/SSi rows (h%2)*64+n, cols J*64 + q*8 + b
        SSr = big.tile([128, NQ * 128], F32)
        SSi = big.tile([128, NQ * 128], F32)
        for h in range(HL):
            q, par = h // 2, h % 2
            ps_s = ps1.tile([128, 128], F32)
            for k in range(2):
                nc.tensor.matmul(ps_s, zft[k][:, h * 128:(h + 1) * 128],
                                 uT[k][:, h * 128:(h + 1) * 128],
                                 start=(k == 0), stop=(k == 1))
            sl = slice(par * 64, (par + 1) * 64)
            dst_r = SSr.rearrange("p (j q b) -> p j q b",
                                  j=T, q=NQ)[sl, :, q, :]
            dst_i = SSi.rearrange("p (j q b) -> p j q b",
                                  j=T, q=NQ)[sl, :, q, :]
            srcv = ps_s.rearrange("p (j b) -> p j b", j=T)
            nc.vector.tensor_copy(dst_r, srcv[0:64])
            nc.scalar.copy(dst_i, srcv[64:128])

        # ---------------- stage 3a: intra-chunk (scan-independent) ---------
        y_stage = big.tile([128, HL * C], F32, tag="ystage")
        for h in range(HL):
            ps_y = psy.tile([128, C], F32)
            nc.tensor.matmul(ps_y, ut0[:, h * 128:(h + 1) * 128],
                             ksh[:, h * C:h * C + C], start=True, stop=False)
            nc.tensor.matmul(ps_y[:, 128:C], ut1[:, h * 128:(h + 1) * 128],
                             ksh[:, h * C:h * C + 128], start=False, stop=True)
            nc.scalar.copy(y_stage[:, h * C:(h + 1) * C], ps_y)

        # ---------------- stage 2: scan over chunks (vector, fp32) ---------
        HSr = big.tile([128, NQ * 128], F32)
        HSi = big.tile([128, NQ * 128], F32)
        nc.vector.memset(HSr, 0.0)
        nc.vector.memset(HSi, 0.0)

        def step_sl(t, I):
            return t[:, I * 64:(I + 1) * 64]

        def ev(t, n=7, off=0):   # [128, n, 64] view of even slices (+off)
            return t.rearrange("p (j2 two x) -> p j2 two x",
                               j2=T // 2, two=2)[:, 0:n, off, :]

        # G[k] = w (.) SS[2k] + SS[2k+1], k = 0..6   (8 ops, [128, 7, 64])
        NK = T // 2 - 1
        Gr = big.tile([128, NK * 64], F32)
        Gi = big.tile([128, NK * 64], F32)
        w8r = war8.rearrange("p (j2 x) -> p j2 x", j2=T // 2)[:, 0:NK, :]
        w8i = wai8.rearrange("p (j2 x) -> p j2 x", j2=T // 2)[:, 0:NK, :]
        Grv = Gr.rearrange("p (k x) -> p k x", k=NK)
        Giv = Gi.rearrange("p (k x) -> p k x", k=NK)
        gm1 = workbig.tile([128, NK * 64], F32, tag="gm1")
        gm1v = gm1.rearrange("p (k x) -> p k x", k=NK)
        gm2 = workbig.tile([128, NK * 64], F32, tag="gm2")
        gm2v = gm2.rearrange("p (k x) -> p k x", k=NK)
        TT(out=gm1v, in0=w8r, in1=ev(SSr), op=MUL)
        TT(out=gm2v, in0=w8i, in1=ev(SSi), op=MUL)
        TT(out=gm1v, in0=gm1v, in1=gm2v, op=SUB)
        TT(out=Grv, in0=gm1v, in1=ev(SSr, off=1), op=ADD)
        TT(out=gm1v, in0=w8i, in1=ev(SSr), op=MUL)
        TT(out=gm2v, in0=w8r, in1=ev(SSi), op=MUL)
        TT(out=gm1v, in0=gm1v, in1=gm2v, op=ADD)
        TT(out=Giv, in0=gm1v, in1=ev(SSi, off=1), op=ADD)

        # even chain: h[2k+2] = w^2 (.) h[2k] + G[k]   (7 steps x 8 ops)
        for k in range(NK):
            pr, pi = step_sl(HSr, 2 * k), step_sl(HSi, 2 * k)
            m1 = work.tile([128, 64], F32, tag="scm1")
            TT(out=m1, in0=w2r, in1=pr, op=MUL)
            m2 = work.tile([128, 64], F32, tag="scm2")
            TT(out=m2, in0=w2i, in1=pi, op=MUL)
            tr = work.tile([128, 64], F32, tag="sctr")
            TT(out=tr, in0=m1, in1=m2, op=SUB)
            TT(out=step_sl(HSr, 2 * k + 2), in0=tr,
               in1=Gr[:, k * 64:(k + 1) * 64], op=ADD)
            m3 = work.tile([128, 64], F32, tag="scm3")
            TT(out=m3, in0=w2i, in1=pr, op=MUL)
            m4 = work.tile([128, 64], F32, tag="scm4")
            TT(out=m4, in0=w2r, in1=pi, op=MUL)
            ti = work.tile([128, 64], F32, tag="scti")
            TT(out=ti, in0=m3, in1=m4, op=ADD)
            TT(out=step_sl(HSi, 2 * k + 2), in0=ti,
               in1=Gi[:, k * 64:(k + 1) * 64], op=ADD)

        # odd fill: h[2k+1] = w (.) h[2k] + SS[2k], all k at once (8 ops)
        w8rf = war8.rearrange("p (j2 x) -> p j2 x", j2=T // 2)
        w8if = wai8.rearrange("p (j2 x) -> p j2 x", j2=T // 2)
        om1 = workbig.tile([128, (T // 2) * 64], F32, tag="om1")
        om1v = om1.rearrange("p (k x) -> p k x", k=T // 2)
        om2 = workbig.tile([128, (T // 2) * 64], F32, tag="om2")
        om2v = om2.rearrange("p (k x) -> p k x", k=T // 2)
        TT(out=om1v, in0=w8rf, in1=ev(HSr, n=T // 2), op=MUL)
        TT(out=om2v, in0=w8if, in1=ev(HSi, n=T // 2), op=MUL)
        TT(out=om1v, in0=om1v, in1=om2v, op=SUB)
        TT(out=ev(HSr, n=T // 2, off=1), in0=om1v,
           in1=ev(SSr, n=T // 2), op=ADD)
        TT(out=om1v, in0=w8if, in1=ev(HSr, n=T // 2), op=MUL)
        TT(out=om2v, in0=w8rf, in1=ev(HSi, n=T // 2), op=MUL)
        TT(out=om1v, in0=om1v, in1=om2v, op=ADD)
        TT(out=ev(HSi, n=T // 2, off=1), in0=om1v,
           in1=ev(SSi, n=T // 2), op=ADD)

        # repack (J, q, b) -> (q, J, b) for stage-3 lhsT slices, cast bf16
        HS2r = big.tile([128, NQ * 128], BF16, tag="hs2r")
        HS2i = big.tile([128, NQ * 128], BF16, tag="hs2i")
        nc.vector.tensor_copy(
            HS2r.rearrange("p (q j b) -> p q j b", q=NQ, j=T),
            HSr.rearrange("p (j q b) -> p q j b", j=T, q=NQ))
        nc.vector.tensor_copy(
            HS2i.rearrange("p (q j b) -> p q j b", q=NQ, j=T),
            HSi.rearrange("p (j q b) -> p q j b", j=T, q=NQ))

        # ---------------- stage 3b: inter-chunk + output -------------------
        W1 = C + 1
        for h in range(HL):
            q, par = h // 2, h % 2
            sl = slice(par * 64, (par + 1) * 64)
            cs = slice(q * 128, (q + 1) * 128)
            ps_y = psy.tile([128, C], F32)
            nc.tensor.matmul(ps_y, HS2r[sl, cs],
                             e2a[sl, q * W1 + 1:q * W1 + C + 1],
                             start=True, stop=False)
            nc.tensor.matmul(ps_y, HS2i[sl, cs],
                             e2b[sl, q * W1 + 1:q * W1 + C + 1],
                             start=False, stop=True)
            TT(out=y_stage[:, h * C:(h + 1) * C],
               in0=y_stage[:, h * C:(h + 1) * C], in1=ps_y, op=ADD)
            if h % 4 == 3:
                nc.sync.dma_start(out=y_d[:, (h - 3) * C:(h + 1) * C],
                                  in_=y_stage[:, (h - 3) * C:(h + 1) * C])

    nc.compile()
    return nc


def _host_prep(u, Lambda_re, Lambda_im, W_ri, D, log_step):
    """Per-core input dicts; all tables in float64 then cast."""
    u = np.asarray(u, np.float32)
    Lr = np.asarray(Lambda_re, np.float64)
    Li = np.asarray(Lambda_im, np.float64)
    W = np.asarray(W_ri, np.float64)
    Dv = np.asarray(D, np.float64)
    ls = np.asarray(log_step, np.float64)

    step = np.exp(ls)                                    # [H]
    a = step[:, None] * Lr                               # [H,N]
    th = step[:, None] * Li
    lam = a + 1j * th                                    # log z
    z = np.exp(lam)
    w = z ** C
    zL = z ** L
    s = (1.0 - zL) / (1.0 - z)
    Lam = Lr + 1j * Li
    c = (W[..., 0] + 1j * W[..., 1]) / Lam * np.conj(s) / (s * np.conj(s) + EPS)

    e = np.arange(C + 1, dtype=np.float64)
    zp = np.exp(lam[..., None] * e)                      # [H,N,C+1]
    cz = c[..., None] * zp                               # [H,N,C+1]
    K = cz.real.sum(axis=1)                              # [H,C+1]

    # Toeplitz index helpers
    p_i = np.arange(128)[:, None]
    m_i = np.arange(C)[None, :]
    dmat = m_i - p_i                                     # [128,C]
    valid = dmat >= 0
    dcl = np.where(valid, dmat, 0)

    pr = np.arange(128)

    in_maps = []
    for core in range(NCORES):
        hs = slice(core * HL, (core + 1) * HL)

        # [B,HL,T,2,128] -> [k][p][h][J][b]
        uu = np.asarray(u[:, hs]).reshape(B, HL, T, 2, 128)
        uu_t = np.ascontiguousarray(
            uu.transpose(3, 4, 1, 2, 0)).reshape(2, 128, T * 128)

        # kshift Toeplitz blocks
        Kh = K[hs]                                       # [HL,C+1]
        M = Kh[:, dcl] * valid[None]                     # [HL,128,C]
        M[:, pr, pr] += Dv[hs, None]
        ksh = np.ascontiguousarray(M.transpose(1, 0, 2)).reshape(128, HL * C)

        # stage-1 weights
        zz = zp[hs]                                      # [HL,N,C+1]
        P0 = zz[:, :, 255 - pr]                          # [HL,N,128]
        P1 = zz[:, :, 127 - pr]
        zft0 = np.stack([P0.real, P0.imag], axis=1)      # [HL,2,N,128]
        zft1 = np.stack([P1.real, P1.imag], axis=1)
        zft0 = np.ascontiguousarray(
            zft0.transpose(3, 0, 1, 2)).reshape(128, HL * 128)
        zft1 = np.ascontiguousarray(
            zft1.transpose(3, 0, 1, 2)).reshape(128, HL * 128)

        # stage-3 tables: [q,par,n,j] -> rows (par,n), cols (q,j)
        czs = cz[hs].reshape(NQ, 2, N, C + 1)
        czt = np.ascontiguousarray(
            czs.transpose(1, 2, 0, 3)).reshape(128, NQ * (C + 1))
        e2a = czt.real
        e2b = -czt.imag

        def wpack(x):  # [HL,N] -> rows (par,n), cols (q,b)
            return np.ascontiguousarray(
                np.asarray(x).reshape(HL // 2, 2, N).transpose(1, 2, 0)
            ).reshape(128, HL // 2, 1).repeat(8, axis=2).reshape(
                128, 64).astype(np.float32)

        wp_r, wp_i = wpack(np.real(w[hs])), wpack(np.imag(w[hs]))
        m = {
            "ut0": uu_t[0].astype(BF16_NP),
            "ut1": uu_t[1].astype(BF16_NP),
            "zft0": zft0.astype(BF16_NP),
            "zft1": zft1.astype(BF16_NP),
            "ksh": ksh.astype(BF16_NP),
            "e2a": e2a.astype(BF16_NP),
            "e2b": e2b.astype(BF16_NP),
            "w2r": wpack(np.real(w[hs] ** 2)),
            "w2i": wpack(np.imag(w[hs] ** 2)),
            "war8": np.tile(wp_r, (1, T // 2)),
            "wai8": np.tile(wp_i, (1, T // 2)),
        }
        in_maps.append(m)
    return in_maps


def _run(inputs, trace=False):
    if "nc" not in _CACHE:
        _CACHE["nc"] = _build()
    nc = _CACHE["nc"]
    in_maps = _host_prep(**inputs)
    res = run_bass_kernel_spmd(nc, in_maps, list(range(NCORES)), trace=trace)
    parts = []
    for core in range(NCORES):
        ys = res.results[core]["y_s"].reshape(T, B, HL, C)
        parts.append(ys.transpose(1, 2, 0, 3).reshape(B, HL, L))
    y = np.concatenate(parts, axis=1)                    # [B, H, L]
    return np.ascontiguousarray(y.astype(np.float32)), res


def kernel(**inputs) -> np.ndarray:
    y, _ = _run(inputs, trace=False)
    return y


def kernel_traced(**inputs):
    y, res = _run(inputs, trace=True)
    return y, res
